# revision 9
# baseline (speedup 1.0000x reference)
"""Trainium2 Bass kernel for out = exp(-M) @ x.

M: [16384, 16384] fp32, x: [16384, 128] fp32 -> out: [16384, 128] fp32.

Sharding: row-shard M and out over 8 cores (2048 rows each), x replicated.

Per-core pipeline (DMA-bound at ~128 MiB HBM reads, ~360 GB/s/core):
  DMA   : M tiles [128, 4096] fp32, natural layout (16 KiB contiguous rows),
          issue alternates SP / ACT sequencers to spread HWDGE setup cost
  DMA   : x loaded once per iteration as 2 SWDGE cast-DMAs (fp32 DRAM ->
          bf16 SBUF, [128 k-part, chunk*feat] layout) on the Pool engine
  ACT   : e = exp(-M_tile) fused fp32 -> bf16 (free affine scale=-1)
  PE    : transpose e chunks [128m, 128k] -> PSUM [128k, 128m] (bf16)
  DVE   : evacuate PSUM -> SBUF rhs tiles [128k, 512m]
  PE    : out.T[f, m] += x[kchunk].T @ rhs   (x stationary bf16, fp32 PSUM acc)
  DVE   : evacuate out.T [f, 512m] PSUM -> SBUF, store via SWDGE; the final
          [m, f] transpose happens on the host (out is returned as [f, m]).

XDB=1 (timing loops only): unroll the repeat loop 2x and double-buffer the
ident/xbf constants so the cross-iteration WAR (x reload waiting on the
previous iteration's last matmul/transpose) never stalls the pipeline.
"""

import os
import sys

sys.path.insert(0, "/opt/trn_rl_repo")

import numpy as np

import concourse.bass as bass  # noqa: F401  (engine namespaces live on nc)
import concourse.mybir as mybir
import concourse.tile as tile
from concourse import bacc
from concourse.bass_utils import run_bass_kernel_spmd
from concourse.masks import make_identity

N = 16384  # M is [N, N]
D = 128  # x is [N, D]
N_CORES = 8
M_ROWS = N // N_CORES  # 2048 rows of M / out per core

F32 = mybir.dt.float32
BF16 = mybir.dt.bfloat16
EXP = mybir.ActivationFunctionType.Exp

# geometry
M_SUPER = 512  # output rows accumulated per PSUM bank
N_SUPERS = M_ROWS // M_SUPER  # 4
K_WIN = int(os.environ.get("KWIN", "4096"))  # contraction window per M DMA tile
N_WINS = N // K_WIN  # 4
M_SUBS = M_SUPER // 128  # 4 m-subtiles per super
KC_PER_WIN = K_WIN // 128  # 32 k-chunks per window
N_KCHUNKS = N // 128  # 128 total k-chunks

XDB = bool(int(os.environ.get("XDB", "1")))
BUFS_M = int(os.environ.get("BUFS_M", "6" if not XDB else "4"))
BUFS_E = int(os.environ.get("BUFS_E", "7"))
SPLIT_DMA = int(os.environ.get("SPLIT_DMA", "1"))
BUFS_PT = int(os.environ.get("BUFS_PT", "5"))
BUFS_RHS = int(os.environ.get("BUFS_RHS", "6"))


def build_kernel(repeats=1, mode="full"):
    nc = bacc.Bacc("TRN2", target_bir_lowering=False, debug=False)
    m_ap = nc.dram_tensor("m_shard", [M_ROWS, N], F32, kind="ExternalInput").ap()
    x_ap = nc.dram_tensor("x", [N, D], F32, kind="ExternalInput").ap()
    # out is stored transposed ([feature, row]); host transposes back.
    out_ap = nc.dram_tensor("out", [D, M_ROWS], F32, kind="ExternalOutput").ap()

    from contextlib import ExitStack

    with tile.TileContext(nc) as tc, ExitStack() as ctx:
        unroll = 2 if (XDB and repeats > 1) else 1
        if repeats > 1:
            assert repeats % unroll == 0
            ctx.enter_context(tc.For_i(0, repeats // unroll, 1))
        consts = ctx.enter_context(tc.tile_pool(name="consts", bufs=1))
        m_pool = ctx.enter_context(tc.tile_pool(name="m", bufs=BUFS_M))
        e_pool = ctx.enter_context(tc.tile_pool(name="e", bufs=BUFS_E))
        rhs_pool = ctx.enter_context(tc.tile_pool(name="rhs", bufs=BUFS_RHS))
        outT_pool = ctx.enter_context(tc.tile_pool(name="outT", bufs=2))
        pt_pool = ctx.enter_context(tc.tile_pool(name="pt", bufs=BUFS_PT, space="PSUM"))
        pout_pool = ctx.enter_context(tc.tile_pool(name="pout", bufs=2, space="PSUM"))

        def body(tag):
            ident_bf = consts.tile([128, 128], BF16, name=f"ident_{tag}")
            make_identity(nc, ident_bf[:])

            # x resident in SBUF as bf16, chunk c at xbf[:, c*D:(c+1)*D]
            # (partition = k within chunk, free = feature).  SWDGE cast-DMA:
            # fp32 DRAM -> bf16 SBUF with the [p, (c f)] access pattern.
            xbf_t = consts.tile([128, N_KCHUNKS * D], BF16, name=f"xbf_{tag}")
            if not os.environ.get("XOFF"):
                x_re = x_ap.rearrange("(c p) f -> p c f", p=128)
                half = N_KCHUNKS // 2
                for h in range(2):
                    nc.gpsimd.dma_start(
                        out=xbf_t[:, h * half * D : (h + 1) * half * D],
                        in_=x_re[:, h * half : (h + 1) * half, :],
                    )

            for ms in range(N_SUPERS):
                pout = (
                    pout_pool.tile([128, M_SUPER], F32, name="pout", tag="pout")
                    if mode not in ("mem", "dma")
                    else None
                )
                outT_mem = (
                    outT_pool.tile([128, M_SUPER], F32, name="outT", tag="outT")
                    if mode in ("mem", "dma")
                    else None
                )
                for kw in range(N_WINS):
                    ebf = []
                    for j in range(M_SUBS):
                        mt = m_pool.tile([128, K_WIN], F32)
                        r0 = ms * M_SUPER + j * 128
                        c0 = kw * K_WIN
                        w = K_WIN // SPLIT_DMA
                        for s in range(SPLIT_DMA):
                            mix = os.environ.get("DMA_MIX", "")
                            idx = j * SPLIT_DMA + s
                            if mix == "hwsw":
                                dma_eng = nc.sync if idx % 2 == 0 else nc.gpsimd
                            elif mix == "3way":
                                dma_eng = (nc.sync, nc.scalar, nc.gpsimd)[idx % 3]
                            elif mix == "sync":
                                dma_eng = nc.sync
                            else:
                                dma_eng = nc.sync if idx % 2 == 0 else nc.scalar
                            dma_eng.dma_start(
                                out=mt[:, s * w : (s + 1) * w],
                                in_=m_ap[r0 : r0 + 128, c0 + s * w : c0 + (s + 1) * w],
                            )
                        if mode == "dma":
                            nc.vector.tensor_copy(
                                outT_mem[:, j * 128 : (j + 1) * 128], mt[:, 0:128]
                            )
                            continue
                        e = e_pool.tile([128, K_WIN], BF16)
                        nc.scalar.activation(e[:], mt[:], EXP, scale=-1.0)
                        ebf.append(e)
                    if mode == "dma":
                        continue
                    if mode == "mem":
                        # probe: DMA + exp only; consume every e tile cheaply
                        for j in range(M_SUBS):
                            nc.vector.tensor_copy(
                                outT_mem[:, j * 128 : (j + 1) * 128], ebf[j][:, 0:128]
                            )
                        continue
                    for kc in range(KC_PER_WIN):
                        kg = kw * KC_PER_WIN + kc
                        if mode == "noT":
                            # probe: skip transposes+copies; feed MM junk rhs
                            off = min(kc * 128, K_WIN - M_SUPER)
                            nc.tensor.matmul(
                                pout[:],
                                lhsT=xbf_t[:, kg * D : (kg + 1) * D],
                                rhs=ebf[0][:, off : off + M_SUPER],
                                start=(kg == 0),
                                stop=(kg == N_KCHUNKS - 1),
                            )
                            continue
                        pt = pt_pool.tile([128, M_SUPER], BF16)
                        for j in range(M_SUBS):
                            nc.tensor.transpose(
                                pt[:, j * 128 : (j + 1) * 128],
                                ebf[j][:, kc * 128 : (kc + 1) * 128],
                                ident_bf[:],
                            )
                        rhs = rhs_pool.tile([128, M_SUPER], BF16)
                        nc.vector.tensor_copy(rhs[:], pt[:])
                        nc.tensor.matmul(
                            pout[:],
                            lhsT=xbf_t[:, kg * D : (kg + 1) * D],
                            rhs=rhs[:],
                            start=(kg == 0),
                            stop=(kg == N_KCHUNKS - 1),
                        )
                # evacuate out.T [f, m-super], store transposed; host fixes
                if mode in ("mem", "dma"):
                    outT = outT_mem
                else:
                    outT = outT_pool.tile([128, M_SUPER], F32)
                    nc.vector.tensor_copy(outT[:], pout[:])
                o_eng = nc.scalar if os.environ.get("OUT_SCALAR") else nc.gpsimd
                o_eng.dma_start(
                    out=out_ap[:, ms * M_SUPER : (ms + 1) * M_SUPER], in_=outT[:]
                )

        for u in range(unroll):
            body("a" if u == 0 else "b")

    nc.compile()
    return nc


_NC_CACHE = None


def _get_nc():
    global _NC_CACHE
    if _NC_CACHE is None:
        _NC_CACHE = build_kernel()
    return _NC_CACHE


def _run_on_device(M, x):
    nc = _get_nc()
    in_maps = [
        {"m_shard": M[c * M_ROWS : (c + 1) * M_ROWS], "x": x} for c in range(N_CORES)
    ]
    res = run_bass_kernel_spmd(nc, in_maps, list(range(N_CORES)))
    return np.concatenate(
        [np.ascontiguousarray(res.results[c]["out"].T) for c in range(N_CORES)], axis=0
    )


def _run_in_subprocess(M, x):
    """Retry path: a fresh process gets a fresh NRT/axon session, which
    recovers from the occasional NRT_EXEC_UNIT_UNRECOVERABLE flake."""
    import subprocess, tempfile

    d = tempfile.mkdtemp(prefix="bassk_")
    np.save(os.path.join(d, "M.npy"), M)
    np.save(os.path.join(d, "x.npy"), x)
    here = os.path.dirname(os.path.abspath(__file__))
    code = (
        "import sys, numpy as np\n"
        f"sys.path.insert(0, {here!r})\n"
        "import kernel\n"
        f"M = np.load({os.path.join(d, 'M.npy')!r})\n"
        f"x = np.load({os.path.join(d, 'x.npy')!r})\n"
        "out = kernel._run_on_device(M, x)\n"
        f"np.save({os.path.join(d, 'out.npy')!r}, out)\n"
    )
    subprocess.run([sys.executable, "-c", code], check=True, timeout=1200)
    return np.load(os.path.join(d, "out.npy"))


def kernel(M, x):
    M = np.ascontiguousarray(np.asarray(M, dtype=np.float32))
    x = np.ascontiguousarray(np.asarray(x, dtype=np.float32))
    assert M.shape == (N, N) and x.shape == (N, D)
    try:
        return _run_on_device(M, x)
    except Exception as e:
        print(f"kernel: in-process run failed ({e!r}); retrying in subprocess",
              file=sys.stderr, flush=True)
    last = None
    for _ in range(2):
        try:
            return _run_in_subprocess(M, x)
        except Exception as e:  # noqa: PERF203
            last = e
    raise last


# revision 10
# speedup vs baseline: 1.0026x; 1.0026x over previous
"""Trainium2 Bass kernel for out = exp(-M) @ x.

M: [16384, 16384] fp32, x: [16384, 128] fp32 -> out: [16384, 128] fp32.

Sharding: row-shard M and out over 8 cores (2048 rows each), x replicated.

Per-core pipeline (DMA-bound at ~128 MiB HBM reads, ~360 GB/s/core):
  DMA   : M tiles [128, 4096] fp32, natural layout (16 KiB contiguous rows),
          issue alternates SP / ACT sequencers to spread HWDGE setup cost
  DMA   : x loaded once per iteration as 2 SWDGE cast-DMAs (fp32 DRAM ->
          bf16 SBUF, [128 k-part, chunk*feat] layout) on the Pool engine
  ACT   : e = exp(-M_tile) fused fp32 -> bf16 (free affine scale=-1)
  PE    : transpose e chunks [128m, 128k] -> PSUM [128k, 128m] (bf16)
  DVE   : evacuate PSUM -> SBUF rhs tiles [128k, 512m]
  PE    : out.T[f, m] += x[kchunk].T @ rhs   (x stationary bf16, fp32 PSUM acc)
  DVE   : evacuate out.T [f, 512m] PSUM -> SBUF, store via SWDGE; the final
          [m, f] transpose happens on the host (out is returned as [f, m]).

XDB=1 (timing loops only): unroll the repeat loop 2x and double-buffer the
ident/xbf constants so the cross-iteration WAR (x reload waiting on the
previous iteration's last matmul/transpose) never stalls the pipeline.
"""

import os
import sys

sys.path.insert(0, "/opt/trn_rl_repo")

import numpy as np

import concourse.bass as bass  # noqa: F401  (engine namespaces live on nc)
import concourse.mybir as mybir
import concourse.tile as tile
from concourse import bacc
from concourse.bass_utils import run_bass_kernel_spmd
from concourse.masks import make_identity

N = 16384  # M is [N, N]
D = 128  # x is [N, D]
N_CORES = 8
M_ROWS = N // N_CORES  # 2048 rows of M / out per core

F32 = mybir.dt.float32
BF16 = mybir.dt.bfloat16
EXP = mybir.ActivationFunctionType.Exp

# geometry
M_SUPER = 512  # output rows accumulated per PSUM bank
N_SUPERS = M_ROWS // M_SUPER  # 4
K_WIN = int(os.environ.get("KWIN", "4096"))  # contraction window per M DMA tile
N_WINS = N // K_WIN  # 4
M_SUBS = M_SUPER // 128  # 4 m-subtiles per super
KC_PER_WIN = K_WIN // 128  # 32 k-chunks per window
N_KCHUNKS = N // 128  # 128 total k-chunks

XDB = bool(int(os.environ.get("XDB", "1")))
BUFS_M = int(os.environ.get("BUFS_M", "6" if not XDB else "4"))
BUFS_E = int(os.environ.get("BUFS_E", "8"))
SPLIT_DMA = int(os.environ.get("SPLIT_DMA", "1"))
BUFS_PT = int(os.environ.get("BUFS_PT", "5"))
BUFS_RHS = int(os.environ.get("BUFS_RHS", "6"))


def build_kernel(repeats=1, mode="full"):
    nc = bacc.Bacc("TRN2", target_bir_lowering=False, debug=False)
    m_ap = nc.dram_tensor("m_shard", [M_ROWS, N], F32, kind="ExternalInput").ap()
    x_ap = nc.dram_tensor("x", [N, D], F32, kind="ExternalInput").ap()
    # out is stored transposed ([feature, row]); host transposes back.
    out_ap = nc.dram_tensor("out", [D, M_ROWS], F32, kind="ExternalOutput").ap()

    from contextlib import ExitStack

    with tile.TileContext(nc) as tc, ExitStack() as ctx:
        unroll = 2 if (XDB and repeats > 1) else 1
        if repeats > 1:
            assert repeats % unroll == 0
            ctx.enter_context(tc.For_i(0, repeats // unroll, 1))
        consts = ctx.enter_context(tc.tile_pool(name="consts", bufs=1))
        m_pool = ctx.enter_context(tc.tile_pool(name="m", bufs=BUFS_M))
        e_pool = ctx.enter_context(tc.tile_pool(name="e", bufs=BUFS_E))
        rhs_pool = ctx.enter_context(tc.tile_pool(name="rhs", bufs=BUFS_RHS))
        outT_pool = ctx.enter_context(tc.tile_pool(name="outT", bufs=2))
        pt_pool = ctx.enter_context(tc.tile_pool(name="pt", bufs=BUFS_PT, space="PSUM"))
        pout_pool = ctx.enter_context(tc.tile_pool(name="pout", bufs=2, space="PSUM"))

        def body(tag):
            ident_bf = consts.tile([128, 128], BF16, name=f"ident_{tag}")
            make_identity(nc, ident_bf[:])

            # x resident in SBUF as bf16, chunk c at xbf[:, c*D:(c+1)*D]
            # (partition = k within chunk, free = feature).  SWDGE cast-DMA:
            # fp32 DRAM -> bf16 SBUF with the [p, (c f)] access pattern.
            xbf_t = consts.tile([128, N_KCHUNKS * D], BF16, name=f"xbf_{tag}")
            if not os.environ.get("XOFF"):
                x_re = x_ap.rearrange("(c p) f -> p c f", p=128)
                half = N_KCHUNKS // 2
                for h in range(2):
                    nc.gpsimd.dma_start(
                        out=xbf_t[:, h * half * D : (h + 1) * half * D],
                        in_=x_re[:, h * half : (h + 1) * half, :],
                    )

            for ms in range(N_SUPERS):
                pout = (
                    pout_pool.tile([128, M_SUPER], F32, name="pout", tag="pout")
                    if mode not in ("mem", "dma")
                    else None
                )
                outT_mem = (
                    outT_pool.tile([128, M_SUPER], F32, name="outT", tag="outT")
                    if mode in ("mem", "dma")
                    else None
                )
                for kw in range(N_WINS):
                    ebf = []
                    for j in range(M_SUBS):
                        mt = m_pool.tile([128, K_WIN], F32)
                        r0 = ms * M_SUPER + j * 128
                        c0 = kw * K_WIN
                        w = K_WIN // SPLIT_DMA
                        for s in range(SPLIT_DMA):
                            mix = os.environ.get("DMA_MIX", "")
                            idx = j * SPLIT_DMA + s
                            if mix == "hwsw":
                                dma_eng = nc.sync if idx % 2 == 0 else nc.gpsimd
                            elif mix == "3way":
                                dma_eng = (nc.sync, nc.scalar, nc.gpsimd)[idx % 3]
                            elif mix == "sync":
                                dma_eng = nc.sync
                            else:
                                dma_eng = nc.sync if idx % 2 == 0 else nc.scalar
                            dma_eng.dma_start(
                                out=mt[:, s * w : (s + 1) * w],
                                in_=m_ap[r0 : r0 + 128, c0 + s * w : c0 + (s + 1) * w],
                            )
                        if mode == "dma":
                            nc.vector.tensor_copy(
                                outT_mem[:, j * 128 : (j + 1) * 128], mt[:, 0:128]
                            )
                            continue
                        e = e_pool.tile([128, K_WIN], BF16)
                        nc.scalar.activation(e[:], mt[:], EXP, scale=-1.0)
                        ebf.append(e)
                    if mode == "dma":
                        continue
                    if mode == "mem":
                        # probe: DMA + exp only; consume every e tile cheaply
                        for j in range(M_SUBS):
                            nc.vector.tensor_copy(
                                outT_mem[:, j * 128 : (j + 1) * 128], ebf[j][:, 0:128]
                            )
                        continue
                    for kc in range(KC_PER_WIN):
                        kg = kw * KC_PER_WIN + kc
                        if mode == "noT":
                            # probe: skip transposes+copies; feed MM junk rhs
                            off = min(kc * 128, K_WIN - M_SUPER)
                            nc.tensor.matmul(
                                pout[:],
                                lhsT=xbf_t[:, kg * D : (kg + 1) * D],
                                rhs=ebf[0][:, off : off + M_SUPER],
                                start=(kg == 0),
                                stop=(kg == N_KCHUNKS - 1),
                            )
                            continue
                        pt = pt_pool.tile([128, M_SUPER], BF16)
                        for j in range(M_SUBS):
                            nc.tensor.transpose(
                                pt[:, j * 128 : (j + 1) * 128],
                                ebf[j][:, kc * 128 : (kc + 1) * 128],
                                ident_bf[:],
                            )
                        rhs = rhs_pool.tile([128, M_SUPER], BF16)
                        nc.vector.tensor_copy(rhs[:], pt[:])
                        nc.tensor.matmul(
                            pout[:],
                            lhsT=xbf_t[:, kg * D : (kg + 1) * D],
                            rhs=rhs[:],
                            start=(kg == 0),
                            stop=(kg == N_KCHUNKS - 1),
                        )
                # evacuate out.T [f, m-super], store transposed; host fixes
                if mode in ("mem", "dma"):
                    outT = outT_mem
                else:
                    outT = outT_pool.tile([128, M_SUPER], F32)
                    nc.vector.tensor_copy(outT[:], pout[:])
                o_eng = nc.scalar if os.environ.get("OUT_SCALAR") else nc.gpsimd
                o_eng.dma_start(
                    out=out_ap[:, ms * M_SUPER : (ms + 1) * M_SUPER], in_=outT[:]
                )

        for u in range(unroll):
            body("a" if u == 0 else "b")

    nc.compile()
    return nc


_NC_CACHE = None


def _get_nc():
    global _NC_CACHE
    if _NC_CACHE is None:
        _NC_CACHE = build_kernel()
    return _NC_CACHE


def _run_on_device(M, x):
    nc = _get_nc()
    in_maps = [
        {"m_shard": M[c * M_ROWS : (c + 1) * M_ROWS], "x": x} for c in range(N_CORES)
    ]
    res = run_bass_kernel_spmd(nc, in_maps, list(range(N_CORES)))
    return np.concatenate(
        [np.ascontiguousarray(res.results[c]["out"].T) for c in range(N_CORES)], axis=0
    )


def _run_in_subprocess(M, x):
    """Retry path: a fresh process gets a fresh NRT/axon session, which
    recovers from the occasional NRT_EXEC_UNIT_UNRECOVERABLE flake."""
    import subprocess, tempfile

    d = tempfile.mkdtemp(prefix="bassk_")
    np.save(os.path.join(d, "M.npy"), M)
    np.save(os.path.join(d, "x.npy"), x)
    here = os.path.dirname(os.path.abspath(__file__))
    code = (
        "import sys, numpy as np\n"
        f"sys.path.insert(0, {here!r})\n"
        "import kernel\n"
        f"M = np.load({os.path.join(d, 'M.npy')!r})\n"
        f"x = np.load({os.path.join(d, 'x.npy')!r})\n"
        "out = kernel._run_on_device(M, x)\n"
        f"np.save({os.path.join(d, 'out.npy')!r}, out)\n"
    )
    subprocess.run([sys.executable, "-c", code], check=True, timeout=1200)
    return np.load(os.path.join(d, "out.npy"))


def kernel(M, x):
    M = np.ascontiguousarray(np.asarray(M, dtype=np.float32))
    x = np.ascontiguousarray(np.asarray(x, dtype=np.float32))
    assert M.shape == (N, N) and x.shape == (N, D)
    try:
        return _run_on_device(M, x)
    except Exception as e:
        print(f"kernel: in-process run failed ({e!r}); retrying in subprocess",
              file=sys.stderr, flush=True)
    last = None
    for _ in range(2):
        try:
            return _run_in_subprocess(M, x)
        except Exception as e:  # noqa: PERF203
            last = e
    raise last


# revision 11
# speedup vs baseline: 1.0391x; 1.0364x over previous
"""Trainium2 Bass kernel for out = exp(-M) @ x.

M: [16384, 16384] fp32, x: [16384, 128] fp32 -> out: [16384, 128] fp32.

Sharding: row-shard M and out over 8 cores (2048 rows each), x replicated.

Per-core pipeline (DMA-bound at ~128 MiB HBM reads, ~360 GB/s/core):
  DMA   : M tiles [128, 4096] fp32, natural layout (16 KiB contiguous rows),
          issue alternates SP / ACT sequencers to spread HWDGE setup cost
  DMA   : x loaded once per iteration as 2 SWDGE cast-DMAs (fp32 DRAM ->
          bf16 SBUF, [128 k-part, chunk*feat] layout) on the Pool engine
  ACT   : e = exp(-M_tile) fused fp32 -> bf16 (free affine scale=-1)
  PE    : transpose e chunks [128m, 128k] -> PSUM [128k, 128m] (bf16)
  DVE   : evacuate PSUM -> SBUF rhs tiles [128k, 512m]
  PE    : out.T[f, m] += x[kchunk].T @ rhs   (x stationary bf16, fp32 PSUM acc)
  DVE   : evacuate out.T [f, 512m] PSUM -> SBUF, store via SWDGE; the final
          [m, f] transpose happens on the host (out is returned as [f, m]).

XDB=1 (timing loops only): unroll the repeat loop 2x and double-buffer the
ident/xbf constants so the cross-iteration WAR (x reload waiting on the
previous iteration's last matmul/transpose) never stalls the pipeline.
"""

import os
import sys

sys.path.insert(0, "/opt/trn_rl_repo")

import numpy as np

import concourse.bass as bass  # noqa: F401  (engine namespaces live on nc)
import concourse.mybir as mybir
import concourse.tile as tile
from concourse import bacc
from concourse.bass_utils import run_bass_kernel_spmd
from concourse.masks import make_identity

N = 16384  # M is [N, N]
D = 128  # x is [N, D]
N_CORES = 8
M_ROWS = N // N_CORES  # 2048 rows of M / out per core

F32 = mybir.dt.float32
BF16 = mybir.dt.bfloat16
EXP = mybir.ActivationFunctionType.Exp

# geometry
M_SUPER = 512  # output rows accumulated per PSUM bank
N_SUPERS = M_ROWS // M_SUPER  # 4
K_WIN = int(os.environ.get("KWIN", "4096"))  # contraction window per M DMA tile
N_WINS = N // K_WIN  # 4
M_SUBS = M_SUPER // 128  # 4 m-subtiles per super
KC_PER_WIN = K_WIN // 128  # 32 k-chunks per window
N_KCHUNKS = N // 128  # 128 total k-chunks

XDB = bool(int(os.environ.get("XDB", "1")))
BUFS_M = int(os.environ.get("BUFS_M", "6" if not XDB else "4"))
BUFS_E = int(os.environ.get("BUFS_E", "7"))
SPLIT_DMA = int(os.environ.get("SPLIT_DMA", "1"))
BUFS_PT = int(os.environ.get("BUFS_PT", "5"))
BUFS_RHS = int(os.environ.get("BUFS_RHS", "6"))


def build_kernel(repeats=1, mode="full"):
    nc = bacc.Bacc("TRN2", target_bir_lowering=False, debug=False)
    m_ap = nc.dram_tensor("m_shard", [M_ROWS, N], F32, kind="ExternalInput").ap()
    x_ap = nc.dram_tensor("x", [N, D], F32, kind="ExternalInput").ap()
    # out is stored transposed ([feature, row]); host transposes back.
    out_ap = nc.dram_tensor("out", [D, M_ROWS], F32, kind="ExternalOutput").ap()

    from contextlib import ExitStack

    with tile.TileContext(nc) as tc, ExitStack() as ctx:
        unroll = 2 if (XDB and repeats > 1) else 1
        if repeats > 1:
            assert repeats % unroll == 0
            ctx.enter_context(tc.For_i(0, repeats // unroll, 1))
        consts = ctx.enter_context(tc.tile_pool(name="consts", bufs=1))
        m_pool = ctx.enter_context(tc.tile_pool(name="m", bufs=BUFS_M))
        e_pool = ctx.enter_context(tc.tile_pool(name="e", bufs=BUFS_E))
        rhs_pool = ctx.enter_context(tc.tile_pool(name="rhs", bufs=BUFS_RHS))
        outT_pool = ctx.enter_context(tc.tile_pool(name="outT", bufs=2))
        pt_pool = ctx.enter_context(tc.tile_pool(name="pt", bufs=BUFS_PT, space="PSUM"))
        pout_pool = ctx.enter_context(tc.tile_pool(name="pout", bufs=2, space="PSUM"))

        def body(tag):
            ident_bf = consts.tile([128, 128], BF16, name=f"ident_{tag}")
            make_identity(nc, ident_bf[:])

            # x resident in SBUF as bf16, chunk c at xbf[:, c*D:(c+1)*D]
            # (partition = k within chunk, free = feature).  SWDGE cast-DMA:
            # fp32 DRAM -> bf16 SBUF with the [p, (c f)] access pattern.
            xbf_t = consts.tile([128, N_KCHUNKS * D], BF16, name=f"xbf_{tag}")
            if not os.environ.get("XOFF"):
                x_re = x_ap.rearrange("(c p) f -> p c f", p=128)
                half = N_KCHUNKS // 2
                for h in range(2):
                    nc.gpsimd.dma_start(
                        out=xbf_t[:, h * half * D : (h + 1) * half * D],
                        in_=x_re[:, h * half : (h + 1) * half, :],
                    )

            for ms in range(N_SUPERS):
                pout = (
                    pout_pool.tile([128, M_SUPER], F32, name="pout", tag="pout")
                    if mode not in ("mem", "dma")
                    else None
                )
                outT_mem = (
                    outT_pool.tile([128, M_SUPER], F32, name="outT", tag="outT")
                    if mode in ("mem", "dma")
                    else None
                )
                for kw in range(N_WINS):
                    ebf = []
                    for j in range(M_SUBS):
                        mt = m_pool.tile([128, K_WIN], F32)
                        r0 = ms * M_SUPER + j * 128
                        c0 = kw * K_WIN
                        w = K_WIN // SPLIT_DMA
                        for s in range(SPLIT_DMA):
                            mix = os.environ.get("DMA_MIX", "")
                            idx = j * SPLIT_DMA + s
                            if mix == "hwsw":
                                dma_eng = nc.sync if idx % 2 == 0 else nc.gpsimd
                            elif mix == "3way":
                                dma_eng = (nc.sync, nc.scalar, nc.gpsimd)[idx % 3]
                            elif mix == "sync":
                                dma_eng = nc.sync
                            else:
                                dma_eng = nc.sync if idx % 2 == 0 else nc.scalar
                            dma_eng.dma_start(
                                out=mt[:, s * w : (s + 1) * w],
                                in_=m_ap[r0 : r0 + 128, c0 + s * w : c0 + (s + 1) * w],
                            )
                        if mode == "dma":
                            nc.vector.tensor_copy(
                                outT_mem[:, j * 128 : (j + 1) * 128], mt[:, 0:128]
                            )
                            continue
                        e = e_pool.tile([128, K_WIN], BF16)
                        nc.scalar.activation(e[:], mt[:], EXP, scale=-1.0)
                        ebf.append(e)
                    if mode == "dma":
                        continue
                    if mode == "mem":
                        # probe: DMA + exp only; consume every e tile cheaply
                        for j in range(M_SUBS):
                            nc.vector.tensor_copy(
                                outT_mem[:, j * 128 : (j + 1) * 128], ebf[j][:, 0:128]
                            )
                        continue
                    for kc in range(KC_PER_WIN):
                        kg = kw * KC_PER_WIN + kc
                        if mode == "noT":
                            # probe: skip transposes+copies; feed MM junk rhs
                            off = min(kc * 128, K_WIN - M_SUPER)
                            nc.tensor.matmul(
                                pout[:],
                                lhsT=xbf_t[:, kg * D : (kg + 1) * D],
                                rhs=ebf[0][:, off : off + M_SUPER],
                                start=(kg == 0),
                                stop=(kg == N_KCHUNKS - 1),
                            )
                            continue
                        pt = pt_pool.tile([128, M_SUPER], BF16)
                        for j in range(M_SUBS):
                            nc.tensor.transpose(
                                pt[:, j * 128 : (j + 1) * 128],
                                ebf[j][:, kc * 128 : (kc + 1) * 128],
                                ident_bf[:],
                            )
                        rhs = rhs_pool.tile([128, M_SUPER], BF16)
                        nc.vector.tensor_copy(rhs[:], pt[:])
                        nc.tensor.matmul(
                            pout[:],
                            lhsT=xbf_t[:, kg * D : (kg + 1) * D],
                            rhs=rhs[:],
                            start=(kg == 0),
                            stop=(kg == N_KCHUNKS - 1),
                        )
                # evacuate out.T [f, m-super], store transposed; host fixes
                if mode in ("mem", "dma"):
                    outT = outT_mem
                else:
                    outT = outT_pool.tile([128, M_SUPER], F32)
                    nc.vector.tensor_copy(outT[:], pout[:])
                o_eng = nc.scalar if os.environ.get("OUT_SCALAR") else nc.gpsimd
                o_eng.dma_start(
                    out=out_ap[:, ms * M_SUPER : (ms + 1) * M_SUPER], in_=outT[:]
                )

        for u in range(unroll):
            body("a" if u == 0 else "b")

    nc.compile()
    return nc


_NC_CACHE = None


def _get_nc():
    global _NC_CACHE
    if _NC_CACHE is None:
        _NC_CACHE = build_kernel()
    return _NC_CACHE


def _run_on_device(M, x):
    nc = _get_nc()
    in_maps = [
        {"m_shard": M[c * M_ROWS : (c + 1) * M_ROWS], "x": x} for c in range(N_CORES)
    ]
    res = run_bass_kernel_spmd(nc, in_maps, list(range(N_CORES)))
    return np.concatenate(
        [np.ascontiguousarray(res.results[c]["out"].T) for c in range(N_CORES)], axis=0
    )


def _run_in_subprocess(M, x):
    """Retry path: a fresh process gets a fresh NRT/axon session, which
    recovers from the occasional NRT_EXEC_UNIT_UNRECOVERABLE flake."""
    import subprocess, tempfile

    d = tempfile.mkdtemp(prefix="bassk_")
    np.save(os.path.join(d, "M.npy"), M)
    np.save(os.path.join(d, "x.npy"), x)
    here = os.path.dirname(os.path.abspath(__file__))
    code = (
        "import sys, numpy as np\n"
        f"sys.path.insert(0, {here!r})\n"
        "import kernel\n"
        f"M = np.load({os.path.join(d, 'M.npy')!r})\n"
        f"x = np.load({os.path.join(d, 'x.npy')!r})\n"
        "out = kernel._run_on_device(M, x)\n"
        f"np.save({os.path.join(d, 'out.npy')!r}, out)\n"
    )
    subprocess.run([sys.executable, "-c", code], check=True, timeout=1200)
    return np.load(os.path.join(d, "out.npy"))


def kernel(M, x):
    M = np.ascontiguousarray(np.asarray(M, dtype=np.float32))
    x = np.ascontiguousarray(np.asarray(x, dtype=np.float32))
    assert M.shape == (N, N) and x.shape == (N, D)
    try:
        return _run_on_device(M, x)
    except Exception as e:
        print(f"kernel: in-process run failed ({e!r}); retrying in subprocess",
              file=sys.stderr, flush=True)
    last = None
    for _ in range(2):
        try:
            return _run_in_subprocess(M, x)
        except Exception as e:  # noqa: PERF203
            last = e
    raise last
